# revision 38
# baseline (speedup 1.0000x reference)
"""AttentionLSTM Trainium2 kernel: data-parallel over batch N across 8 cores.

Each core processes N_loc=32 batch rows through the full T=128 recurrence.
No collectives needed. Matmuls in bf16 (fp32 PSUM accumulate), attention +
gates in fp32 on DVE/ACT. Softmax exp computed via tanh identity so only one
activation table set (sigmoid/tanh) is ever loaded.

Self-contained: only imports concourse (installed) + numpy.
"""
import numpy as np
from contextlib import ExitStack

import concourse.bass as bass
import concourse.tile as tile
from concourse import bacc, mybir, bass_utils
from concourse.bass import ts
from concourse.masks import make_identity

F32 = mybir.dt.float32
BF16 = mybir.dt.bfloat16
AF = mybir.ActivationFunctionType
OP = mybir.AluOpType

N, T, D, H = 256, 128, 1024, 1024
H4 = 4 * H
NC = 8
NL = N // NC          # 32 batch rows per core
KT = H // 128         # 8 k-tiles per 1024 contraction
NJ = H4 // 512        # 8 output col tiles
JORD = [0, 2, 4, 6, 1, 7, 3, 5]  # j order: tail-critical gates last
SCALE = 1.0 / 32.0    # 1/sqrt(H)


def build_nc(t_steps=T, xw_chunks=None):
    nc = bacc.Bacc("TRN2", target_bir_lowering=False, debug=False)
    # x fed pre-transposed by host: (KT, 128, T, NL) = x.transpose(2,1,0)
    x_ext = nc.dram_tensor("x", [KT, 128, T, NL], F32, kind="ExternalInput").ap()
    a_ext = nc.dram_tensor("A", [NL, H, 16], F32, kind="ExternalInput").ap()
    wx_ext = nc.dram_tensor("Wx", [D, H4], F32, kind="ExternalInput").ap()
    wh_ext = nc.dram_tensor("Wh", [H, H4], F32, kind="ExternalInput").ap()
    wa_ext = nc.dram_tensor("Wattn", [H, H4], F32, kind="ExternalInput").ap()
    b_ext = nc.dram_tensor("b", [1, H4], F32, kind="ExternalInput").ap()
    out_ext = nc.dram_tensor("out", [NL, T, H], F32, kind="ExternalOutput").ap()

    with tile.TileContext(nc) as tc, ExitStack() as ctx:
        # ---------- persistent pools ----------
        const_pool = ctx.enter_context(tc.tile_pool(name="const", bufs=1))
        state_pool = ctx.enter_context(tc.tile_pool(name="state", bufs=1))
        xw_dram_pool = ctx.enter_context(
            tc.tile_pool(name="xwdram", bufs=1, space="DRAM"))

        # identity replicated at each 32-partition block
        ident_rep = const_pool.tile([128, 32], F32)
        for g in range(4):
            make_identity(nc, ident_rep[ts(g, 32), :])
        ident128b = const_pool.tile([128, 128], BF16)
        make_identity(nc, ident128b[:])

        # persistent state
        h = state_pool.tile([NL, H], F32)          # h_t
        cst = state_pool.tile([NL, H], F32)        # c_t
        h_pack = state_pool.tile([128, 256], BF16)  # (g*32+n, h') packing
        lhsT = state_pool.tile([128, 16, NL], BF16)  # [hT (k<8); attnT (k>=8)]
        acc = state_pool.tile([128, 256], F32)      # attn accumulator
        af_pack = state_pool.tile([128, 16, 256], BF16)

        xw_dram = xw_dram_pool.tile([T, NL, H4], BF16)

        # ================= PHASE A: XW = x @ Wx + b, Af repack, h0 ========
        with ExitStack() as actx:
            aconst_pool = actx.enter_context(tc.tile_pool(name="aconst", bufs=1))
            wx_pool = actx.enter_context(tc.tile_pool(name="wx", bufs=1))
            stage_pool = actx.enter_context(tc.tile_pool(name="stage", bufs=1))
            xchunk_pool = actx.enter_context(tc.tile_pool(name="xchunk", bufs=3))
            xt_pool = actx.enter_context(tc.tile_pool(name="xt", bufs=2))
            xw_sb_pool = actx.enter_context(tc.tile_pool(name="xwsb", bufs=3))
            ps_xw_pool = actx.enter_context(
                tc.tile_pool(name="psxw", bufs=2, space="PSUM"))
            afst_pool = actx.enter_context(tc.tile_pool(name="afst", bufs=2))

            ones_bf = aconst_pool.tile([1, 128], BF16)
            nc.vector.memset(ones_bf[:], 1.0)
            b_bf = aconst_pool.tile([1, H4], BF16)

            # load Wx -> bf16
            wxs = wx_pool.tile([128, KT, H4], BF16)
            for k in range(KT):
                wstage = stage_pool.tile([128, H4], F32, tag="wstage")
                nc.sync.dma_start(wstage[:], wx_ext[ts(k, 128), :])
                nc.vector.tensor_copy(wxs[:, k, :], wstage[:])
            # b -> bf16
            bstage = stage_pool.tile([1, H4], F32, tag="wstage")
            nc.sync.dma_start(bstage[:], b_ext)
            nc.vector.tensor_copy(b_bf[:], bstage[:])

            # Af (per 256-col group): h0 partial + af_pack
            for g in range(4):
                afg = afst_pool.tile([NL, 256, 16], F32)
                nc.sync.dma_start(afg[:], a_ext[:, ts(g, 256), :])
                nc.vector.tensor_reduce(h[:, ts(g, 256)], afg[:],
                                        mybir.AxisListType.X, OP.add)
                nc.vector.tensor_copy(af_pack[ts(g, 32), :, :],
                                      afg[:].rearrange("n h l -> n l h"))
            nc.scalar.mul(h[:], h[:], 1.0 / 16.0)
            nc.vector.tensor_copy(cst[:], h[:])

            # XW chunks: 4 timesteps x 32 batch = 128 rows each
            for ck in range((t_steps + 3) // 4 if xw_chunks is None else xw_chunks):
                xt = xt_pool.tile([128, KT, 128], BF16)
                for k in range(KT):
                    xc = xchunk_pool.tile([128, 128], F32)
                    nc.sync.dma_start(
                        xc[:],
                        x_ext[k, :, ts(ck, 4), :].rearrange("d t n -> d (t n)"))
                    nc.vector.tensor_copy(xt[:, k, :], xc[:])
                for j in range(NJ):
                    ps = ps_xw_pool.tile([128, 512], F32)
                    for k in range(KT):
                        nc.tensor.matmul(ps[:], xt[:, k, :],
                                         wxs[:, k, ts(j, 512)],
                                         start=(k == 0), stop=False)
                    nc.tensor.matmul(ps[:], ones_bf[:],
                                     b_bf[:, ts(j, 512)],
                                     start=False, stop=True)
                    xw_sb = xw_sb_pool.tile([128, 512], BF16)
                    if j % 2 == 0:
                        nc.vector.tensor_copy(xw_sb[:], ps[:])
                    else:
                        nc.scalar.copy(xw_sb[:], ps[:])
                    nc.sync.dma_start(
                        xw_dram[ts(ck, 4), :, ts(j, 512)].rearrange(
                            "t n c -> (t n) c"),
                        xw_sb[:])

        # ================= PHASE B: load Wh/Wattn, recurrence =============
        wh_pool = ctx.enter_context(tc.tile_pool(name="wh", bufs=1))
        whs = wh_pool.tile([128, KT, H4], BF16)
        was = wh_pool.tile([128, KT, H4], BF16)
        with ExitStack() as bctx:
            stage2 = bctx.enter_context(tc.tile_pool(name="stage2", bufs=2))
            for k in range(KT):
                wstage = stage2.tile([128, H4], F32, tag="w2")
                nc.sync.dma_start(wstage[:], wh_ext[ts(k, 128), :])
                nc.vector.tensor_copy(whs[:, k, :], wstage[:])
            for k in range(KT):
                wstage = stage2.tile([128, H4], F32, tag="w2")
                nc.sync.dma_start(wstage[:], wa_ext[ts(k, 128), :])
                nc.vector.tensor_copy(was[:, k, :], wstage[:])

        # recurrence pools
        sm_pool = ctx.enter_context(tc.tile_pool(name="sm", bufs=1))
        xw_pool = ctx.enter_context(tc.tile_pool(name="xwin", bufs=2))
        gate_pool = ctx.enter_context(tc.tile_pool(name="gate", bufs=1))
        scr_pool = ctx.enter_context(tc.tile_pool(name="scr", bufs=1))
        ps_pre_pool = ctx.enter_context(
            tc.tile_pool(name="pspre", bufs=6, space="PSUM"))
        ps_t_pool = ctx.enter_context(
            tc.tile_pool(name="pst2", bufs=2, space="PSUM"))

        def emit_h_derivatives(h_ap):
            """h (32,1024) f32 -> h_pack bf16 + lhsT[:, 0:8, :] bf16."""
            for k in range(KT):
                pst = ps_t_pool.tile([128, NL], F32, tag="pst")
                nc.tensor.transpose(pst[:], h_ap[:, ts(k, 128)],
                                    ident_rep[0:32, :])
                nc.scalar.copy(lhsT[:, k, :], pst[:])
            for g in range(4):
                nc.vector.tensor_copy(h_pack[ts(g, 32), :],
                                      h_ap[:, ts(g, 256)])

        emit_h_derivatives(h[:])

        for t in range(t_steps):
            # prefetch XW_t
            xw_t = xw_pool.tile([NL, H4], BF16)
            nc.sync.dma_start(xw_t[:], xw_dram[t])

            # ---- logits on DVE (needs h_pack of t-1) ----
            smA = sm_pool.tile([128, 32], F32, tag="smA")
            smB = sm_pool.tile([32, 132], F32, tag="smB")
            stt_scr = scr_pool.tile([128, 256], BF16, tag="sttscr")
            lp = smA[:, 0:16]
            for l in range(16):
                nc.vector.scalar_tensor_tensor(
                    out=stt_scr[:], in0=af_pack[:, l, :], scalar=SCALE,
                    in1=h_pack[:], op0=OP.mult, op1=OP.mult,
                    accum_out=lp[:, l:l + 1])

            # ---- h-part matmuls for j=0..5 run during the attention chain;
            #      the tiny logits block-sum MM is slotted after j=2 so the
            #      softmax isn't stuck behind all six of them in the PE FIFO
            ps_pre = []
            for j in range(NJ):
                psj = ps_pre_pool.tile([NL, 512], F32, tag="ps")
                ps_pre.append(psj)
            ps_lg = ps_t_pool.tile([32, 16], F32, tag="pst")
            # k-halves: k=0..3 MMs can start as soon as the first half of
            #  hT (written in the previous step's tail) is available
            for jj, j in enumerate(JORD[:6]):
                for k in range(4):
                    nc.tensor.matmul(ps_pre[j][:], lhsT[:, k, :],
                                     whs[:, k, ts(j, 512)],
                                     start=(k == 0), stop=False)
            for jj, j in enumerate(JORD[:6]):
                for k in range(4, KT):
                    nc.tensor.matmul(ps_pre[j][:], lhsT[:, k, :],
                                     whs[:, k, ts(j, 512)],
                                     start=False, stop=False)
                if jj == 2:
                    nc.tensor.matmul(ps_lg[:], ident_rep[:], lp[:],
                                     start=True, stop=True)
            nc.scalar.activation(smB[:, 48:64], ps_lg[:], AF.Tanh, scale=0.5)
            nc.vector.tensor_scalar(smB[:, 80:96], smB[:, 48:64],
                                    -1.0, 1.0, OP.mult, OP.add)
            nc.vector.reciprocal(smB[:, 96:112], smB[:, 80:96])
            nc.vector.tensor_scalar(smB[:, 112:128], smB[:, 96:112],
                                    2.0, -1.0, OP.mult, OP.add)
            nc.vector.tensor_reduce(smB[:, 128:129], smB[:, 112:128],
                                    mybir.AxisListType.X, OP.add)
            nc.vector.reciprocal(smB[:, 129:130], smB[:, 128:129])
            w_rep = smA[:, 16:32]
            nc.vector.tensor_scalar(w_rep[0:32, :], smB[:, 112:128],
                                    smB[:, 129:130], None, OP.mult)
            for g in range(1, 4):
                nc.scalar.copy(w_rep[ts(g, 32), :], w_rep[0:32, :])

            # ---- attn = sum_l Af[:,l,:] * w_l (single chain) ----
            nc.vector.tensor_scalar(acc[:], af_pack[:, 0, :],
                                    w_rep[:, 0:1], None, OP.mult)
            for l in range(1, 16):
                nc.vector.scalar_tensor_tensor(
                    out=acc[:], in0=af_pack[:, l, :],
                    scalar=w_rep[:, l:l + 1], in1=acc[:],
                    op0=OP.mult, op1=OP.add)
            for g in range(4):
                for hf2 in range(2):
                    k = g * 2 + hf2
                    pst = ps_t_pool.tile([128, NL], F32, tag="pst")
                    nc.tensor.transpose(
                        pst[:], acc[ts(g, 32), ts(hf2, 128)],
                        ident_rep[ts(g, 32), :], tile_position=(g * 32, 0))
                    nc.scalar.copy(lhsT[:, 8 + k, :], pst[:])
            for j in JORD[:6]:
                for k in range(KT):
                    nc.tensor.matmul(ps_pre[j][:], lhsT[:, 8 + k, :],
                                     was[:, k, ts(j, 512)],
                                     start=False, stop=(k == KT - 1))

            # ---- gates; j order [0,2,4,6,1,3,5,7]: first 512 cols of all
            #      four gates finish early so the c/h chain for cols 0:512
            #      overlaps the remaining matmuls ----
            si = gate_pool.tile([NL, H], F32, tag="si")
            sf = gate_pool.tile([NL, H], F32, tag="sf")
            so = gate_pool.tile([NL, H], F32, tag="so")
            t2 = gate_pool.tile([NL, H], F32, tag="t2")
            gdst = [si, si, sf, sf, so, so, t2, t2]
            gfun = [AF.Sigmoid] * 6 + [AF.Tanh] * 2

            def finish_j(j):
                nc.vector.scalar_tensor_tensor(
                    out=ps_pre[j][:], in0=ps_pre[j][:], scalar=1.0,
                    in1=xw_t[:, ts(j, 512)], op0=OP.mult, op1=OP.add)
                nc.scalar.activation(gdst[j][:, ts(j % 2, 512)], ps_pre[j][:],
                                     gfun[j])

            for j in JORD[:6]:
                finish_j(j)
            for j in JORD[6:]:
                for k in range(KT):
                    nc.tensor.matmul(ps_pre[j][:], lhsT[:, k, :],
                                     whs[:, k, ts(j, 512)],
                                     start=(k == 0), stop=False)
                for k in range(KT):
                    nc.tensor.matmul(ps_pre[j][:], lhsT[:, 8 + k, :],
                                     was[:, k, ts(j, 512)],
                                     start=False, stop=(k == KT - 1))
                finish_j(j)

            # ---- c/h update + h derivatives, per 512-col half ----
            for hf in range(2):
                sl = ts(hf, 512)
                nc.vector.tensor_mul(t2[:, sl], si[:, sl], t2[:, sl])
                nc.vector.tensor_mul(cst[:, sl], sf[:, sl], cst[:, sl])
                nc.vector.tensor_add(cst[:, sl], cst[:, sl], t2[:, sl])
                nc.scalar.activation(t2[:, sl], cst[:, sl], AF.Tanh)
                nc.vector.tensor_mul(h[:, sl], so[:, sl], t2[:, sl])
                for k in range(4 * hf, 4 * hf + 4):
                    pst = ps_t_pool.tile([128, NL], F32, tag="pst")
                    nc.tensor.transpose(pst[:], h[:, ts(k, 128)],
                                        ident_rep[0:32, :])
                    nc.scalar.copy(lhsT[:, k, :], pst[:])
                for g in (0, 1) if hf == 0 else (2, 3):
                    nc.vector.tensor_copy(h_pack[ts(g, 32), :],
                                          h[:, ts(g, 256)])
            nc.sync.dma_start(out_ext[:, t, :], h[:])

    nc.compile()
    return nc


_cached = None


def _get_nc():
    global _cached
    if _cached is None:
        _cached = build_nc()
    return _cached


def kernel(**inputs):
    x = np.asarray(inputs["x"], dtype=np.float32)
    A = np.ascontiguousarray(
        np.asarray(inputs["A"], dtype=np.float32).reshape(N, H, 16))
    Wx = np.ascontiguousarray(np.asarray(inputs["Wx"], dtype=np.float32))
    Wh = np.ascontiguousarray(np.asarray(inputs["Wh"], dtype=np.float32))
    Wattn = np.ascontiguousarray(np.asarray(inputs["Wattn"], dtype=np.float32))
    b = np.ascontiguousarray(
        np.asarray(inputs["b"], dtype=np.float32).reshape(1, H4))

    nc = _get_nc()
    in_maps = []
    for c in range(NC):
        sl = slice(c * NL, (c + 1) * NL)
        xp = np.ascontiguousarray(
            x[sl].transpose(2, 1, 0)).reshape(KT, 128, T, NL)
        in_maps.append({
            "x": xp, "A": A[sl], "Wx": Wx, "Wh": Wh, "Wattn": Wattn,
            "b": b,
        })
    res = bass_utils.run_bass_kernel_spmd(nc, in_maps, core_ids=list(range(NC)))
    out = np.concatenate([res.results[c]["out"] for c in range(NC)], axis=0)
    return out.astype(np.float32)


if __name__ == "__main__":
    rng = np.random.default_rng(0)
    ins = {
        "x": rng.standard_normal((N, T, D), dtype=np.float32),
        "A": rng.standard_normal((N, H, 4, 4), dtype=np.float32),
        "Wx": rng.standard_normal((D, H4), dtype=np.float32) / 32,
        "Wh": rng.standard_normal((H, H4), dtype=np.float32) / 32,
        "Wattn": rng.standard_normal((H, H4), dtype=np.float32) / 32,
        "b": np.zeros((H4,), dtype=np.float32),
    }
    out = kernel(**ins)
    print("ran:", out.shape, out.dtype)
